# revision 2
# baseline (speedup 1.0000x reference)
"""HBV (HBVMulTDET) Trainium2 Bass kernel, v4.

v2 -> v3 (trace-driven):
  - v2's Ln/Exp on the Activation engine caused an ACT_TABLE_LOAD (~1.5us)
    every step (Ln and Exp live in different default act tables) = 2.25ms of
    the 4.8ms runtime. v3 computes soil wetness sm^BETA with the GpSimd
    engine's tensor_tensor(pow) instead: no ACT instructions remain at all.
  - Custom fused DVE ops (registered at import): WR = win*(1-swe);
    SOILA = max(min(SMb,1)*petA, F0); SOILB = max(min(SMb,1)-pet, F0);
    SUZF = u3 - K0*relu(u3-1); RXS = rech + relu(SMb-1)  (wide).
  - Upper zone runs in UZL-scaled units so the Q0 threshold is the
    immediate 1.0, which lets SUZF fuse threshold+relu+mult+sub.
Per step: snow 7 DVE, soil 1 GpSimd(pow) + 6 DVE, upper zone 5 DVE.
SLZ exact via tensor_tensor_scan; outputs assembled wide per chunk.
"""

import math
import os
import sys

import numpy as np

for _p in ("/opt/trn_rl_repo",):
    if _p not in sys.path:
        sys.path.insert(0, _p)

T_FULL, G, NM = 730, 4000, 8
NCORES = 8
GL = G // NCORES
P = 125
GSUB = GL // P
FW = GSUB * NM
NZ = 1e-5
F0 = 1e-8

BOUNDS = np.array([[1.0, 6.0], [50.0, 1000.0], [0.05, 0.9], [0.01, 0.5],
                   [0.001, 0.2], [0.2, 1.0], [0.0, 10.0], [0.0, 100.0],
                   [-2.5, 2.5], [0.5, 10.0], [0.0, 0.1], [0.0, 0.2]],
                  dtype=np.float32)

_CONSTS = ["BETA", "NCWH", "K0", "K1C", "invFC"]
NCONST = len(_CONSTS)
# wide const order: AB (1-K2), AUZ ((1-K2)*UZLS), FCoUZ (FC/UZLS), C2, UZLS
_WCONSTS = ["AB", "AUZ", "FCoUZ", "C2", "UZLS", "PCSB"]
NWIDE = len(_WCONSTS)

_PROGRAM_CACHE = {}
LAST_RESULTS = None

# ---- custom DVE ops (registered into concourse.dve_ops at import) ----------


def _register_custom_ops():
    from concourse.dve_spec import Spec, Src0, Src1, relu, lower, One, C0, maxx, minn
    from concourse.dve_uop import DveOpSpec
    from concourse import dve_ops

    def reg(name, spec):
        for o in dve_ops.OPS:
            if o.name == name:
                return o
        shas = {}
        for ver in ("v3", "v4"):
            u = lower(spec, ver=ver)
            shas[ver] = DveOpSpec(name=name, uops=u,
                                  rd1_en=dve_ops.has_src1(spec)).sha(ver)
        op = dve_ops.DveOp(name, spec, subdim=False, uops_sha=shas)
        dve_ops.OPS.append(op)
        dve_ops.CUSTOM_DVE_SPECS[name] = spec
        dve_ops._SUB_OPCODE_FOR_NAME[name] = (
            dve_ops._CUSTOM_DVE_ROW_BASE + len(dve_ops.OPS) - 1
        )
        return op

    ops = {}
    ops["WR"] = reg("HBV_WR_ANT", Spec(
        body=Src0 * (One - Src1),
        reference=lambda in0, in1, s0, s1, imm2: in0 * (1.0 - in1)))
    ops["SOILA"] = reg("HBV_SOILA_ANT", Spec(
        body=maxx(minn(Src0, One) * Src1, C0),
        reference=lambda in0, in1, s0, s1, imm2: np.maximum(
            np.minimum(in0, 1.0) * in1, s0)))
    ops["SOILB"] = reg("HBV_SOILB_ANT", Spec(
        body=maxx(minn(Src0, One) - Src1, C0),
        reference=lambda in0, in1, s0, s1, imm2: np.maximum(
            np.minimum(in0, 1.0) - in1, s0)))
    ops["SUZF"] = reg("HBV_SUZF_ANT", Spec(
        body=Src0 - Src1 * relu(Src0 - One),
        reference=lambda in0, in1, s0, s1, imm2: in0 - in1 * np.maximum(
            in0 - 1.0, 0.0)))
    ops["RXS"] = reg("HBV_RXS_ANT", Spec(
        body=Src0 + relu(Src1 - One),
        reference=lambda in0, in1, s0, s1, imm2: in0 + np.maximum(
            in1 - 1.0, 0.0)))
    ops["RELU2"] = reg("HBV_RELU2_ANT", Spec(
        body=relu(Src0 + Src1),
        reference=lambda in0, in1, s0, s1, imm2: np.maximum(in0 + in1, 0.0)))
    return ops


_OPS = None


def _get_ops():
    global _OPS
    if _OPS is None:
        _OPS = _register_custom_ops()
    return _OPS


def _build_program(t_steps, C):
    import concourse.bass as bass
    import concourse.bacc as bacc
    import concourse.mybir as mybir
    import concourse.tile as tile
    from contextlib import ExitStack

    f32 = mybir.dt.float32
    Alu = mybir.AluOpType
    Act = mybir.ActivationFunctionType
    OPS = _get_ops()

    # Pin Ln/Exp/Relu to the single table set that holds all three, so the
    # compiler emits one ACT_TABLE_LOAD instead of one per Ln<->Exp switch.
    if not getattr(bacc, "_hbv_act_tables_patched", False):
        _orig_tables = bacc.get_activation_tables

        def _patched_tables(arch):
            keep = "natural_log_exp_and_others"
            strip = {Act.Ln, Act.Exp, Act.Relu}
            out = {}
            for name, s in _orig_tables(arch).items():
                out[name] = set(s) if name == keep else set(s) - strip
            return out

        bacc.get_activation_tables = _patched_tables
        bacc._hbv_act_tables_patched = True

    nc = bacc.Bacc()

    d_snow = nc.dram_tensor("snow", [P, t_steps * FW], f32, kind="ExternalInput")
    d_phi = nc.dram_tensor("phi", [P, t_steps * FW], f32, kind="ExternalInput")
    d_rain = nc.dram_tensor("rain", [P, t_steps * FW], f32, kind="ExternalInput")
    d_pet = nc.dram_tensor("pet", [P, t_steps * FW], f32, kind="ExternalInput")
    d_petA = nc.dram_tensor("petA", [P, t_steps * FW], f32, kind="ExternalInput")
    d_const = nc.dram_tensor("consts", [P, NCONST * FW], f32, kind="ExternalInput")
    d_constw = nc.dram_tensor("constsw", [P, NWIDE * C * FW], f32, kind="ExternalInput")
    d_q = nc.dram_tensor("q", [P, t_steps * FW], f32, kind="ExternalOutput")

    NCk = math.ceil(t_steps / C)

    def clen(n):
        return min(C, t_steps - n * C)

    VE, GE = nc.vector, nc.gpsimd

    with ExitStack() as ctx:
        tc = ctx.enter_context(tile.TileContext(nc))
        cpool = ctx.enter_context(tc.tile_pool(name="consts", bufs=1))
        spool = ctx.enter_context(tc.tile_pool(name="state", bufs=2))
        tpool = ctx.enter_context(tc.tile_pool(name="temps", bufs=2))
        ipool2 = ctx.enter_context(tc.tile_pool(name="inputs2", bufs=2))
        ipool3 = ctx.enter_context(tc.tile_pool(name="inputs3", bufs=2))
        r2 = ctx.enter_context(tc.tile_pool(name="series", bufs=2))
        r3 = ctx.enter_context(tc.tile_pool(name="series3", bufs=3))

        ct = cpool.tile([P, NCONST * FW], f32)
        nc.sync.dma_start(ct[:], d_const[:, :])
        K = {name: ct[:, i * FW:(i + 1) * FW] for i, name in enumerate(_CONSTS)}
        ctw = cpool.tile([P, NWIDE * C * FW], f32)
        nc.sync.dma_start(ctw[:], d_constw[:, :])
        W = {name: ctw[:, i * C * FW:(i + 1) * C * FW]
             for i, name in enumerate(_WCONSTS)}

        # --- state init ---
        m0 = cpool.tile([P, FW], f32, name="m0")
        mneg = cpool.tile([P, FW], f32, name="mneg")
        VE.memset(m0[:], 0.001)
        VE.memset(mneg[:], -0.001)
        SP = spool.tile([P, FW], f32, tag="SP", name="SP")
        VE.tensor_mul(SP[:], m0[:], K["invFC"])
        SM = spool.tile([P, FW], f32, tag="SM", name="SM")
        VE.tensor_mul(SM[:], m0[:], K["invFC"])
        NMW0 = spool.tile([P, FW], f32, tag="NMW", name="NMW")
        VE.tensor_mul(NMW0[:], mneg[:], K["invFC"])
        # SUZ in UZL-scaled units: 0.001/UZLS = 0.001*invFC*FCoUZ... simpler:
        # host packs invUZLS into consts? use m0 * (FC/UZLS) * invFC:
        t0_ = cpool.tile([P, FW], f32, name="t0_")
        VE.tensor_mul(t0_[:], m0[:], K["invFC"])
        SUZ0 = spool.tile([P, FW], f32, tag="SUZ", name="SUZ")
        VE.tensor_mul(SUZ0[:], t0_[:], W["FCoUZ"][:, :FW])
        st = {"SP": SP, "NMW": NMW0[:], "SM": SM, "SUZ": SUZ0[:], "slz_last": None}
        ch = [dict() for _ in range(NCk)]

        def sl(t):
            return slice(t * FW, (t + 1) * FW)

        def dma_in(n, names, pool_):
            cw = clen(n) * FW
            cols = slice(n * C * FW, n * C * FW + cw)
            tensors = {"snow": d_snow, "phi": d_phi, "rain": d_rain,
                       "pet": d_pet, "petA": d_petA}
            for nm in names:
                tl = pool_.tile([P, cw], f32, tag=nm, name=f"{nm}{n}")
                nc.sync.dma_start(tl[:], tensors[nm][:, cols])
                ch[n][nm] = tl

        def tp(tag):
            return tpool.tile([P, FW], f32, tag=tag, name=tag)

        def passA(n):  # snow
            cn = clen(n)
            snow, phi = ch[n]["snow"], ch[n]["phi"]
            NM2s = r2.tile([P, cn * FW], f32, tag="NM2s", name=f"NM2s{n}")
            NMns = r2.tile([P, cn * FW], f32, tag="NMns", name=f"NMns{n}")
            ch[n]["NM2s"], ch[n]["NMns"] = NM2s, NMns
            for t in range(cn):
                s = sl(t)
                SP1 = tp("SP1")
                VE.tensor_add(SP1[:], st["SP"][:], snow[:, s])
                mx = tp("mx")
                VE.tensor_max(mx[:], phi[:, s], st["NMW"])
                net = tp("net")
                VE.tensor_tensor(net[:], mx[:], SP1[:], Alu.min)
                SPn = spool.tile([P, FW], f32, tag="SP", name="SP")
                VE.tensor_sub(SPn[:], SP1[:], net[:])
                VE.tensor_sub(NM2s[:, s], st["NMW"], net[:])
                ncw = tp("ncw")
                VE.tensor_mul(ncw[:], K["NCWH"], SPn[:])
                VE.tensor_max(NMns[:, s], NM2s[:, s], ncw[:])
                st["SP"], st["NMW"] = SPn, NMns[:, s]

        def passW1(n):  # win = rain + (NMns - NM2s)
            cn = clen(n)
            w0 = r2.tile([P, cn * FW], f32, tag="w0", name=f"w0{n}")
            VE.tensor_sub(w0[:], ch[n]["NMns"][:], ch[n]["NM2s"][:])
            win = r3.tile([P, cn * FW], f32, tag="win", name=f"win{n}")
            VE.tensor_add(win[:], ch[n]["rain"][:], w0[:])
            ch[n]["win"] = win

        def passB(n):  # soil: Ln/Exp on ACT (single pinned table) + fused DVE
            cn = clen(n)
            win, pet, petA = ch[n]["win"], ch[n]["pet"], ch[n]["petA"]
            AE = nc.scalar
            swes = r2.tile([P, cn * FW], f32, tag="swes", name=f"swes{n}")
            SMbs = r2.tile([P, cn * FW], f32, tag="SMbs", name=f"SMbs{n}")
            ch[n]["swes"], ch[n]["SMbs"] = swes, SMbs
            for t in range(cn):
                s = sl(t)
                lsm = tp("lsm")
                AE.activation(lsm[:], st["SM"][:], Act.Ln)
                e1 = tp("e1")
                VE.tensor_mul(e1[:], K["BETA"], lsm[:])
                AE.activation(swes[:, s], e1[:], Act.Exp)
                wr = tp("wr")
                VE._custom_dve(OPS["WR"], out=wr[:], in0=win[:, s], in1=swes[:, s])
                VE.tensor_add(SMbs[:, s], st["SM"][:], wr[:])
                c1 = tp("c1")
                VE._custom_dve(OPS["SOILA"], out=c1[:], in0=SMbs[:, s],
                               in1=petA[:, s], s0=F0)
                c2 = tp("c2")
                VE._custom_dve(OPS["SOILB"], out=c2[:], in0=SMbs[:, s],
                               in1=pet[:, s], s0=F0)
                SMn = spool.tile([P, FW], f32, tag="SM", name="SM")
                VE.tensor_max(SMn[:], c1[:], c2[:])
                st["SM"] = SMn

        def passWb(n):  # rech = win*swe; rx = (rech+relu(SMb-1))*FC/UZLS; rxp = rx-PCS
            cn = clen(n)
            cw = cn * FW
            rech = r2.tile([P, cw], f32, tag="rech", name=f"rech{n}")
            VE.tensor_mul(rech[:], ch[n]["win"][:], ch[n]["swes"][:])
            rxs = r2.tile([P, cw], f32, tag="rxs", name=f"rxs{n}")
            VE._custom_dve(OPS["RXS"], out=rxs[:], in0=rech[:],
                           in1=ch[n]["SMbs"][:])
            rx = r3.tile([P, cw], f32, tag="rx", name=f"rx{n}")
            VE.tensor_mul(rx[:], rxs[:], W["FCoUZ"][:, :cw])
            rxp = r2.tile([P, cw], f32, tag="rxp", name=f"rxp{n}")
            VE.tensor_sub(rxp[:], rx[:], W["PCSB"][:, :cw])
            ch[n]["rx"], ch[n]["rxp"] = rx, rxp

        def passC(n):  # upper zone (UZL-scaled): 3 fused ops/step
            cn = clen(n)
            rxp = ch[n]["rxp"]
            u3s = r2.tile([P, cn * FW], f32, tag="u3s", name=f"u3s{n}")
            uns = r2.tile([P, cn * FW], f32, tag="uns", name=f"uns{n}")
            ch[n]["u3s"], ch[n]["uns"] = u3s, uns
            ch[n]["un_bound"] = st["SUZ"]
            for t in range(cn):
                s = sl(t)
                VE._custom_dve(OPS["RELU2"], out=u3s[:, s], in0=st["SUZ"],
                               in1=rxp[:, s])
                u4 = tp("u4")
                VE._custom_dve(OPS["SUZF"], out=u4[:], in0=u3s[:, s], in1=K["K0"])
                VE.tensor_mul(uns[:, s], K["K1C"], u4[:])
                st["SUZ"] = uns[:, s]

        def passWd(n):  # perc recovery + SLZ scan + output assembly
            cn = clen(n)
            cw = cn * FW
            rx = ch[n]["rx"]
            u2w = r2.tile([P, cw], f32, tag="w0", name=f"u2w{n}")
            VE.tensor_add(u2w[:, :FW], ch[n]["un_bound"], rx[:, :FW])
            if cn > 1:
                VE.tensor_add(u2w[:, FW:], ch[n]["uns"][:, :cw - FW], rx[:, FW:])
            percw = r2.tile([P, cw], f32, tag="NM2s", name=f"percw{n}")
            VE.tensor_sub(percw[:], u2w[:], ch[n]["u3s"][:])
            d1 = r2.tile([P, cw], f32, tag="d1", name=f"d1{n}")
            VE.tensor_mul(d1[:], W["AUZ"][:, :cw], percw[:])
            slzs = r2.tile([P, cw], f32, tag="slzs", name=f"slzs{n}")
            prev = st["slz_last"]
            for e in range(FW):
                init = 0.001 if prev is None else prev[:, (prev.shape[1] - FW + e):(prev.shape[1] - FW + e + 1)]
                VE.tensor_tensor_scan(slzs[:, e::FW], W["AB"][:, e:cw:FW],
                                      d1[:, e::FW], init, Alu.mult, Alu.add)
            st["slz_last"] = slzs[:]
            quw = r2.tile([P, cw], f32, tag="quw", name=f"quw{n}")
            VE.tensor_sub(quw[:], ch[n]["u3s"][:], ch[n]["uns"][:])
            qu = r2.tile([P, cw], f32, tag="quw", name=f"qu{n}")
            VE.tensor_mul(qu[:], quw[:], W["UZLS"][:, :cw])
            q2 = r2.tile([P, cw], f32, tag="d1", name=f"q2{n}")
            VE.tensor_mul(q2[:], W["C2"][:, :cw], slzs[:])
            qout = r2.tile([P, cw], f32, tag="qout", name=f"qout{n}")
            VE.tensor_add(qout[:], qu[:], q2[:])
            cols = slice(n * C * FW, n * C * FW + cw)
            nc.sync.dma_start(d_q[:, cols], qout[:])
            ch[n].clear()

        for p in range(NCk + 5):
            if p < NCk:
                dma_in(p, ("snow", "phi", "rain"), ipool2)
            if 0 <= p - 1 < NCk:
                dma_in(p - 1, ("pet", "petA"), ipool3)
            if 0 <= p - 1 < NCk:
                passA(p - 1)
                passW1(p - 1)
            if 0 <= p - 2 < NCk:
                passB(p - 2)
            if 0 <= p - 3 < NCk:
                passWb(p - 3)
            if 0 <= p - 4 < NCk:
                passC(p - 4)
            if 0 <= p - 5 < NCk:
                passWd(p - 5)

    nc.finalize()
    return nc


def _to_kernel_layout(a, t_steps):
    return np.ascontiguousarray(
        a.reshape(t_steps, P, GSUB, NM).transpose(1, 0, 2, 3).reshape(P, t_steps * FW)
    )


def _from_kernel_layout(a, t_steps):
    return a.reshape(P, t_steps, GSUB, NM).transpose(1, 0, 2, 3).reshape(t_steps, GL, NM)


def _const_layout(c):
    return np.ascontiguousarray(
        c.reshape(P, GSUB, NM).reshape(P, FW)
    ).astype(np.float32)


def prepare_in_maps(x, pr, t_steps, C):
    x = np.asarray(x, dtype=np.float32)
    pr = np.asarray(pr, dtype=np.float32)

    b = BOUNDS
    p = pr[-1] * (b[:, 1] - b[:, 0])[None, :, None] + b[:, 0][None, :, None]
    (BETA, FC, K0, K1, K2, LP, PERCc, UZL, TT, CFMAX, CFR, CWH) = (
        p[:, i, :] for i in range(12)
    )
    CFRX = CFR * CFMAX
    f64 = np.float64
    invFC = (1.0 / FC.astype(f64)).astype(np.float32)
    invLP = (1.0 / LP.astype(f64)).astype(np.float32)
    K1C = (1.0 - K1.astype(f64)).astype(np.float32)
    A2 = (1.0 - K2.astype(f64)).astype(np.float32)
    C2 = (K2.astype(f64) / (1.0 - K2.astype(f64))).astype(np.float32)
    UZLS = np.maximum(UZL, 1e-6).astype(np.float32)
    PCS = (PERCc.astype(f64) / UZLS.astype(f64)).astype(np.float32)
    FCoUZ = (FC.astype(f64) / UZLS.astype(f64)).astype(np.float32)
    AUZ = (A2.astype(f64) * UZLS.astype(f64)).astype(np.float32)

    in_maps = []
    for k in range(NCORES):
        cs = slice(k * GL, (k + 1) * GL)
        prcp = x[:t_steps, cs, 0]
        tmean = x[:t_steps, cs, 1]
        pet = x[:t_steps, cs, 2]
        dT = tmean[:, :, None] - TT[None, cs, :]
        is_rain = (dT >= 0).astype(np.float32)
        RAIN = prcp[:, :, None] * is_rain
        SNOW = prcp[:, :, None] - RAIN
        PHI = CFMAX[None, cs, :] * np.maximum(dT, 0.0) - CFRX[None, cs, :] * np.maximum(-dT, 0.0)
        iFC = invFC[None, cs, :]
        snow_s = (SNOW * iFC).astype(np.float32)
        phi_s = (PHI * iFC).astype(np.float32)
        rain_s = (RAIN * iFC).astype(np.float32)
        pet_s = (pet[:, :, None] * iFC).astype(np.float32)
        petA = (1.0 - pet_s * invLP[None, cs, :]).astype(np.float32)

        consts = np.stack([
            _const_layout(BETA[cs]), _const_layout(-CWH[cs]),
            _const_layout(K0[cs]), _const_layout(K1C[cs]),
            _const_layout(invFC[cs]),
        ], axis=1).reshape(P, NCONST * FW)

        wideconsts = np.stack([
            np.tile(_const_layout(A2[cs]), (1, C)),
            np.tile(_const_layout(AUZ[cs]), (1, C)),
            np.tile(_const_layout(FCoUZ[cs]), (1, C)),
            np.tile(_const_layout(C2[cs]), (1, C)),
            np.tile(_const_layout(UZLS[cs]), (1, C)),
            np.tile(_const_layout(PCS[cs]), (1, C)),
        ], axis=1).reshape(P, NWIDE * C * FW)

        in_maps.append({
            "snow": _to_kernel_layout(snow_s, t_steps),
            "phi": _to_kernel_layout(phi_s, t_steps),
            "rain": _to_kernel_layout(rain_s, t_steps),
            "pet": _to_kernel_layout(pet_s, t_steps),
            "petA": _to_kernel_layout(petA, t_steps),
            "consts": np.ascontiguousarray(consts, dtype=np.float32),
            "constsw": np.ascontiguousarray(wideconsts, dtype=np.float32),
        })
    return in_maps


def kernel(x_hydro_model, params_raw, t_steps=None):
    global LAST_RESULTS
    from concourse.bass_utils import run_bass_kernel_spmd

    if t_steps is None:
        t_steps = int(x_hydro_model.shape[0])
    C = int(os.environ.get("HBV_CHUNK", "32"))
    in_maps = prepare_in_maps(x_hydro_model, params_raw, t_steps, C)

    key = (t_steps, C)
    if key not in _PROGRAM_CACHE:
        _PROGRAM_CACHE[key] = _build_program(t_steps, C)
    nc = _PROGRAM_CACHE[key]

    res = run_bass_kernel_spmd(nc, in_maps, core_ids=list(range(NCORES)))
    LAST_RESULTS = res

    out = np.concatenate(
        [_from_kernel_layout(res.results[k]["q"], t_steps) for k in range(NCORES)],
        axis=1,
    )
    return out.astype(np.float32)
